# revision 45
# baseline (speedup 1.0000x reference)
"""ACT (Adaptive Computation Time) pondering network on 8 trn2 NeuronCores.

Data-parallel: 16384 positions sharded 2048/core; weights replicated.
All loop state SBUF-resident; fp32 matmuls (exact halting decisions vs the
f32 reference); halting chain replicates the reference's f32 op order.

Layout: activations transposed to [D, positions] so weights are matmul-
stationary and biases are per-partition scalars.

Toolchain constraint: each instruction may carry at most ONE semaphore wait
(+ its own update). All cross-engine deps funnel through DVE; startup
"observer" ops absorb DMA-queue semaphore ticks per engine.
"""
from contextlib import ExitStack
import numpy as np

import concourse.bass as bass
import concourse.tile as tile
from concourse import mybir
from concourse.bass_utils import run_bass_kernel_spmd

AF = mybir.ActivationFunctionType
OP = mybir.AluOpType
F32 = mybir.dt.float32


class SplitDrainTileContext(tile.TileContext):
    """Tile's kernel-tail drain collects one wait per proc (17 here) on a
    single instruction; this walrus build only encodes a couple of sync
    commands per instruction. Split the waits across standalone SP nops."""

    def _drain_and_barrier(self, tick_clock, wait_clock):
        from concourse.vector_clock import ScopedClock

        carrier = self.nc.sync.nop(nofuse=True)
        wait_clock.add_sem_waits(
            carrier.ins, ScopedClock({None: tick_clock.global_clock})
        )
        si = carrier.ins.sync_info
        waits = list(si.on_wait) if si and si.on_wait else []
        if len(waits) > 1:
            carrier.ins.sync_info = mybir.SyncInfo(
                on_wait=[waits[0]], on_update=list(si.on_update or [])
            )
            for w in waits[1:]:
                n = self.nc.sync.nop(nofuse=True)
                n.ins.sync_info = mybir.SyncInfo(on_wait=[w], on_update=[])
        # the nops above already hold SP until every proc reaches its final
        # tick, so the drain itself needs no waits
        self.nc.sync.drain()
        self.nc.all_engine_barrier()
        assert self.sems is not None
        popped = self.nc._tile_sem_poison_stack.pop()
        assert popped is self._sem_poison
        self.nc.clear_and_free_semaphores(list(self.sems.allocated().values()))
        self.nc.all_engine_barrier()

NCORES = 8
B, L, D, F = 4, 4096, 512, 2048
POS = B * L // NCORES          # 2048 positions per core
DC = D // 128                  # 4 d-chunks
FC = F // 128                  # 16 f-chunks
CH = 512                       # position chunk (one PSUM bank of f32)
PC = POS // CH                 # 4 position chunks
T = 11                         # MAX_ITERATIONS + 1
THR = float(np.float32(1.0 - 0.01))


def build_graph(bp0: float, n_iters: int = T):
    nc = bass.Bass()
    # const AP for the sigmoid bias (preamble, like Bass.__init__'s 0.0/1.0)
    _bp = nc.alloc_sbuf_tensor("const-bp0", [128, 1], F32)
    nc.gpsimd.memset(_bp.ap(), bp0)
    nc.const_aps.aps[(F32, bp0)] = _bp.ap()
    nc.all_engine_barrier()

    s0_ext = nc.declare_dram_parameter("s0", [DC, 128, POS], F32, isOutput=False)
    w1_ext = nc.declare_dram_parameter("w1", [DC, 128, F], F32, isOutput=False)
    w2_ext = nc.declare_dram_parameter("w2", [FC, 128, D], F32, isOutput=False)
    co_ext = nc.declare_dram_parameter("consts", [128, 688], F32, isOutput=False)
    mask_ext = nc.declare_dram_parameter("maskr", [PC, CH], F32, isOutput=False)
    ost_ext = nc.declare_dram_parameter("out_state", [DC, 128, POS], F32, isOutput=True)
    onu_ext = nc.declare_dram_parameter("out_nup", [PC, CH], F32, isOutput=True)
    ore_ext = nc.declare_dram_parameter("out_rem", [PC, CH], F32, isOutput=True)

    with SplitDrainTileContext(nc) as tc:
        with (
            tc.tile_pool(name="big", bufs=1) as big,
            tc.tile_pool(name="rows", bufs=1) as rows,
            tc.tile_pool(name="h1p", bufs=16) as h1p,
            tc.tile_pool(name="hwp", bufs=3) as hwp,
            tc.tile_pool(name="ps1", bufs=2, space="PSUM") as ps1,
            tc.tile_pool(name="ps2", bufs=2, space="PSUM") as ps2,
            tc.tile_pool(name="psp", bufs=2, space="PSUM") as psp,
            tc.tile_pool(name="psb", bufs=2, space="PSUM") as psb,
        ):
            # ---- persistent tiles + loads (one dma_start per tile) ----
            s = [big.tile([128, POS], F32, name=f"s{dc}", tag=f"s{dc}") for dc in range(DC)]
            w1 = [big.tile([128, F], F32, name=f"w1_{dc}", tag=f"w1_{dc}") for dc in range(DC)]
            w2 = [big.tile([128, D], F32, name=f"w2_{fc}", tag=f"w2_{fc}") for fc in range(FC)]
            prev = [big.tile([128, POS], F32, name=f"prev{dc}", tag=f"prev{dc}") for dc in range(DC)]
            uwb = big.tile([128, POS], F32, name="uwb", tag="uwb")
            # small constants share one host-packed tile: b1 @0:16, b2 @16:20,
            # emb @20:64, eye @64:576 (partitions 0..3), wpc @576:640,
            # scratch @640:688
            consts = rows.tile([128, 688], F32, name="consts", tag="consts")
            b1 = consts[:, 0:FC]
            b2 = consts[:, FC:FC + DC]
            _embo = FC + DC
            # halting-state rows live as [PC, CH] tiles (partition = position
            # chunk): every engine access starts at partition 0, and
            # two-input DVE ops see equal base partitions.
            def _r(name):
                return rows.tile([PC, CH], F32, name=name, tag=name)
            uw, hp, rem, nup = _r("uw"), _r("hp"), _r("rem"), _r("nup")
            pd, still, ps_ = _r("pd"), _r("still"), _r("ps_")
            q, nh, st2, t1 = _r("q"), _r("nh"), _r("st2"), _r("t1")
            prow_t = [_r("prow0"), _r("prow1"), _r("prow2")]  # rotates t%3
            maskr = _r("maskr")
            # aliases onto rows that are dead by the time these are written
            t3, u, u2 = still, q, ps_
            zrow = t1  # t1 fully consumed before the next iteration's z lands
            sacd = consts[0:1, 640:644]


            for dc in range(DC):
                nc.sync.dma_start(s[dc][:], s0_ext[dc])
                nc.sync.dma_start(w1[dc][:], w1_ext[dc])
            for fc in range(FC):
                nc.sync.dma_start(w2[fc][:], w2_ext[fc])
            nc.sync.dma_start(consts[:], co_ext[:])
            nc.sync.dma_start(maskr[:], mask_ext[:])


            nc.vector.memset(hp[:], 0.0)
            nc.vector.memset(rem[:], 0.0)
            nc.vector.memset(nup[:], 0.0)
            for dc in range(DC):
                nc.vector.memset(prev[dc][:], 0.0)

            # ---- startup observers: absorb DMA-queue sem ticks per engine ----
            # PE reads a 1x2 sliver of every matmul input so later matmuls
            # never need a DMA wait on top of their DVE wait.
            sac = psp.tile([1, 16], F32, name="sac", tag="pp")
            for tl in (*s, *w1, *w2, consts):
                nc.tensor.matmul(sac[:, 0:2], tl[0:1, 0:1], tl[0:1, 0:2],
                                 start=True, stop=True)
            # DVE touches the bias/mask tiles it will read mid-loop.
            nc.vector.tensor_copy(sacd[:, 0:1], b1[0:1, 0:1])
            nc.vector.tensor_copy(sacd[:, 1:2], maskr[0:1, 0:1])
            # ACT observes the consts DMA once so the per-iteration pre-reads
            # (which write into consts scratch) carry only the DVE wait.
            nc.scalar.copy(consts[0:1, 642:643], consts[0:1, 0:1])

            # guard registers: one set, reloaded each guarded iteration
            GUARD_FROM = 2
            # per-iteration alive scratch: unique addresses avoid Pool WAW waits
            # loop scratch lives in its own (never-DMA'd) tile so post-If
            # clock forks can't resurrect DMA-queue waits on its readers
            scr = rows.tile([4, 64], F32, name="scr", tag="scr")
            # reg-load targets live in their own tile: TensorLoad dependency
            # tracking is coarse, so writes to shared scratch would WAR them
            gscr = rows.tile([1, 32], F32, name="gscr", tag="gscr")
            def galive_w(tt):
                return gscr[0:1, tt:tt + 1]
            ones4 = scr[0:4, 50:51]
            cnt4 = scr[0:4, 48:49]

            nc.vector.memset(ones4, 1.0)
            # DVE observes the s-tile DMA queues (first DVE write to s is now
            # the epilogue s_next op, which must carry only the PE wait)
            for dc in range(DC):
                nc.vector.tensor_copy(scr[0:1, 44 + dc:45 + dc], s[dc][0:1, 0:1])
            regs = nc.alloc_registers(
                "alv", bass.OrderedSet([mybir.EngineType.PE, mybir.EngineType.DVE,
                                        mybir.EngineType.Activation]))

            # ---- the 11 ACT iterations ----
            TAIL_FROM = 5
            def emit_uw_bcast(t):
                # alive total + per-chunk counts for next iteration's guards
                pal = psb.tile([1, 1], F32, name=f"pal_{t}", tag="pb")
                nc.tensor.matmul(pal[:], ones4, cnt4, start=True, stop=True)
                nc.vector.tensor_copy(galive_w(t), pal[:])
                if t == TAIL_FROM - 2:
                    # per-chunk active counts feeding t+1's chunk guards;
                    # cnt4 (post-update) = exactly who participates at t+1
                    pcr = psb.tile([1, PC], F32, name=f"pcr_{t}", tag="pb")
                    nc.tensor.matmul(pcr[:], cnt4, consts[0:PC, 680:680 + PC],
                                     start=True, stop=True)
                    nc.vector.tensor_copy(gscr[0:1, 16:16 + PC], pcr[:])
                # broadcast uw row j across partitions via eye-matmul
                for j in range(PC):
                    pb = psb.tile([128, CH], F32, name=f"pb_{t}_{j}", tag="pb")
                    nc.tensor.matmul(
                        pb[:], consts[0:PC, 64 + j * 128:64 + (j + 1) * 128],
                        uw[:], start=True, stop=True)
                    nc.vector.tensor_copy(uwb[:, j * CH:(j + 1) * CH], pb[:])

            def iter_body(t, absorb, chunk_guard=False):
                with nc.named_scope(f"iter{t}"):
                    if absorb:
                        # body-entry absorber: after an If, engine clocks fork
                        # conservatively; give DVE its ACT observation in one
                        # single-wait op before real work
                        nc.vector.tensor_copy(scr[0:1, 32 + t:33 + t],
                                              prow_t[(t - 1) % 3][0:1, 0:1])
                    # pondering: z = s . Wp  (fp32 exact): one [PC, CH] psum,
                    # row j from masked-Wp columns against position chunk j
                    pp = psp.tile([PC, CH], F32, name=f"pp_{t}", tag="pp")
                    for pc in range(PC):
                        for dc in range(DC):
                            wcol = 576 + (pc * DC + dc) * 4
                            nc.tensor.matmul(pp[:], consts[:, wcol:wcol + 4],
                                             s[dc][:, pc * CH:(pc + 1) * CH],
                                             start=(pc == 0 and dc == 0),
                                             stop=(pc == PC - 1 and dc == DC - 1))
                    nc.vector.tensor_copy(zrow[:], pp[:])
                    prow = prow_t[t % 3]
                    # ACT pre-read of one zrow element into a fresh scratch
                    # address: absorbs the DVE wait so the sigmoid carries only
                    # its own-engine (prow WAW) wait.
                    _sc = 644 + t
                    nc.scalar.copy(consts[0:1, _sc:_sc + 1], zrow[0:1, 0:1])
                    nc.scalar.activation(prow[:], zrow[:], AF.Sigmoid,
                                         bias=bp0, scale=1.0)
                    # halting chain; replicates reference f32 op order exactly
                    nc.vector.tensor_copy(pd[:], prow[:])          # import p to DVE
                    nc.vector.tensor_scalar(still[:], hp[:], 1.0, None, OP.is_lt)
                    nc.vector.tensor_mul(ps_[:], pd[:], still[:])
                    nc.vector.tensor_add(q[:], hp[:], ps_[:])
                    nc.vector.tensor_scalar(nh[:], q[:], THR, None, OP.is_gt)
                    nc.vector.tensor_mul(nh[:], nh[:], still[:])
                    nc.vector.tensor_sub(st2[:], still[:], nh[:])
                    # alive count for the next iteration's guard:
                    # DVE free-dim reduce, PE ones-matmul across the 4 chunks,
                    # DVE copy back -- each op carries one wait
                    nc.vector.tensor_reduce(cnt4, st2[:], axis=mybir.AxisListType.X,
                                            op=OP.add)
                    nc.vector.tensor_mul(t1[:], pd[:], st2[:])
                    nc.vector.tensor_add(hp[:], hp[:], t1[:])
                    nc.vector.tensor_scalar(u[:], hp[:], -1.0, 1.0, OP.mult, OP.add)
                    nc.vector.tensor_mul(u2[:], nh[:], u[:])
                    nc.vector.tensor_add(rem[:], rem[:], u2[:])
                    nc.vector.tensor_mul(t3[:], nh[:], rem[:])
                    nc.vector.tensor_add(hp[:], hp[:], t3[:])
                    nc.vector.tensor_add(nup[:], nup[:], st2[:])
                    nc.vector.tensor_add(nup[:], nup[:], nh[:])
                    nc.vector.tensor_add(uw[:], t1[:], t3[:])
                    nc.vector.tensor_mul(uw[:], uw[:], maskr[:])
                    if chunk_guard:
                        # per-chunk counts were computed at t-1; chunks with
                        # zero active positions contribute nothing (uw==0
                        # there), so skipping them is exact
                        emit_uw_bcast(t)
                    # FFN per position chunk
                    for pc in range(PC):
                        cstk = ExitStack()
                        if chunk_guard:
                            for reg in regs:
                                nc.reg_load(reg, gscr[0:1, 16 + pc:17 + pc]
                                            .bitcast(mybir.dt.int32))
                            cstk.enter_context(tc.If(nc.snap(regs) > 0, preferred_fallthrough_block=True))
                            # chunk-entry absorber: re-observe DVE inside the
                            # forked block with a single-wait op
                            nc.vector.tensor_copy(scr[0:1, 56 + pc:57 + pc],
                                                  uwb[0:1, pc * CH:pc * CH + 1])
                        sl = slice(pc * CH, (pc + 1) * CH)
                        h1 = []
                        for fc in range(FC):
                            pm = ps1.tile([128, CH], F32, name=f"pm_{t}_{pc}_{fc}", tag="pm")
                            for dc in range(DC):
                                nc.tensor.matmul(pm[:], w1[dc][:, fc * 128:(fc + 1) * 128],
                                                 s[dc][:, sl],
                                                 start=(dc == 0), stop=(dc == DC - 1))
                            h1t = h1p.tile([128, CH], F32, name=f"h1_{t}_{pc}_{fc}", tag="h1")
                            # h1 = max(pm + b1, 0)  (exact IEEE on DVE)
                            nc.vector.tensor_scalar(h1t[:], pm[:], b1[:, fc:fc + 1], 0.0,
                                                    OP.add, OP.max)
                            h1.append(h1t)
                        if pc == 0 and not chunk_guard:
                            # emitted here so these PE ops sit BEHIND the first
                            # mm1 block: the DVE halting chain they depend on
                            # finishes while mm1 streams
                            emit_uw_bcast(t)
                        for dt in range(DC):
                            pm2 = ps2.tile([128, CH], F32, name=f"pm2_{t}_{pc}_{dt}", tag="pm2")
                            for fc in range(FC):
                                nc.tensor.matmul(pm2[:], w2[fc][:, dt * 128:(dt + 1) * 128],
                                                 h1[fc][:], start=(fc == 0), stop=(fc == FC - 1))
                            if True:
                                # s_next first: the next iteration's pondering
                                # depends on it, so clear that edge early
                                ec = _embo + dt * T + min(t + 1, T - 1)
                                nc.vector.tensor_scalar(s[dt][:, sl], pm2[:],
                                                        b2[:, dt:dt + 1],
                                                        consts[:, ec:ec + 1],
                                                        OP.add, OP.add)
                            # prev += (pm2 + b2) * uw; h split from the product so
                            # each op carries a single semaphore wait; half-width
                            # temps to fit SBUF
                            for hf in range(2):
                                hsl = slice(pc * CH + hf * (CH // 2),
                                            pc * CH + (hf + 1) * (CH // 2))
                                psl = slice(hf * (CH // 2), (hf + 1) * (CH // 2))
                                ht = hwp.tile([128, CH // 2], F32,
                                              name=f"h_{t}_{pc}_{dt}_{hf}", tag="ht")
                                nc.vector.tensor_scalar(ht[:], pm2[:, psl],
                                                        b2[:, dt:dt + 1], None, OP.add)
                                hw = hwp.tile([128, CH // 2], F32,
                                              name=f"hw_{t}_{pc}_{dt}_{hf}", tag="hw")
                                nc.vector.tensor_mul(hw[:], ht[:], uwb[:, hsl])
                                nc.vector.tensor_add(prev[dt][:, hsl],
                                                     prev[dt][:, hsl], hw[:])
                        cstk.close()

            # unguarded warmup iterations, individually guarded middle, then
            # one guard over the whole tail (alive is monotone; a stale-true
            # tail just runs exact no-op iterations)
            for t in range(min(GUARD_FROM, n_iters)):
                iter_body(t, absorb=False)
            for t in range(GUARD_FROM, min(TAIL_FROM, n_iters)):
                with ExitStack() as stk:
                    for reg in regs:
                        # positive f32 bit patterns order like positive ints
                        nc.reg_load(reg, galive_w(t - 1).bitcast(mybir.dt.int32))
                    stk.enter_context(tc.If(nc.snap(regs) > 0, preferred_fallthrough_block=True))
                    iter_body(t, absorb=True, chunk_guard=(t == TAIL_FROM - 1))
            if n_iters > TAIL_FROM:
                with ExitStack() as stk:
                    for reg in regs:
                        nc.reg_load(reg, galive_w(TAIL_FROM - 1).bitcast(mybir.dt.int32))
                    stk.enter_context(tc.If(nc.snap(regs) > 0, preferred_fallthrough_block=True))
                    for t in range(TAIL_FROM, n_iters):
                        iter_body(t, absorb=(t == TAIL_FROM))

            # ---- outputs (gpsimd SWDGE: fresh queues, one wait each) ----
            for dc in range(DC):
                nc.gpsimd.dma_start(ost_ext[dc], prev[dc][:])
            nc.gpsimd.dma_start(onu_ext[:], nup[:])
            nc.gpsimd.dma_start(ore_ext[:], rem[:])

    return nc, tc


def check_waits(nc, verbose=True):
    """Static check: no instruction may carry more than one semaphore wait."""
    bad = 0
    for bb in nc.m.functions[0].blocks:
        for i in bb.instructions:
            si = i.sync_info
            nw = len(si.on_wait) if si and si.on_wait else 0
            if nw >= 2:
                bad += 1
                if verbose and bad <= 12:
                    print(f"MULTI-WAIT {type(i).__name__} {i.name}")
                    for w in si.on_wait:
                        print("   W:", str(w)[:100])
    return bad


def prepare_in_maps(inputs):
    state = np.asarray(inputs["state"], np.float32).reshape(-1, D)
    mask = np.asarray(inputs["mask"], np.float32).reshape(-1)
    emb = np.asarray(inputs["step_emb"], np.float32)
    Wp = np.asarray(inputs["Wp"], np.float32)
    W1 = np.asarray(inputs["W1"], np.float32)
    b1 = np.asarray(inputs["b1"], np.float32)
    W2 = np.asarray(inputs["W2"], np.float32)
    b2 = np.asarray(inputs["b2"], np.float32)

    w1t = np.ascontiguousarray(W1.reshape(DC, 128, F))
    w2t = np.ascontiguousarray(W2.reshape(FC, 128, D))

    # consts block: b1 @0:16, b2 @16:20, emb @20:64 (col = dt*T + t),
    # eye @64:576 (partition pc has ones in cols [pc*128,(pc+1)*128)),
    # wpc @576:640 (block (pc*DC+dc): col m==pc gets Wp[dc*128+k]),
    # scratch @640:688
    co = np.zeros((128, 688), np.float32)
    co[:, 0:FC] = b1.reshape(FC, 128).T
    co[:, FC:FC + DC] = b2.reshape(DC, 128).T
    co[:, FC + DC:FC + DC + DC * T] = emb.T.reshape(DC, 128, T).transpose(1, 0, 2).reshape(128, DC * T)
    for pc in range(PC):
        co[pc, 64 + pc * 128:64 + (pc + 1) * 128] = 1.0
    for pc in range(PC):
        for dc in range(DC):
            co[:, 576 + (pc * DC + dc) * 4 + pc] = Wp[dc * 128:(dc + 1) * 128, 0]
    co[0:4, 680:684] = np.eye(4, dtype=np.float32)

    in_maps = []
    for c in range(NCORES):
        shard = state[c * POS:(c + 1) * POS]                     # [POS, D]
        s0 = (shard.T + emb[0][:, None]).astype(np.float32)      # s_0 = state + emb[0]
        in_maps.append({
            "s0": np.ascontiguousarray(s0.reshape(DC, 128, POS)),
            "w1": w1t, "w2": w2t, "consts": co,
            "maskr": np.ascontiguousarray(mask[c * POS:(c + 1) * POS].reshape(PC, CH)),
        })
    return in_maps


def postprocess(results):
    st_parts, nup_parts, rem_parts = [], [], []
    for r in results:
        st_parts.append(r["out_state"].reshape(D, POS).T)        # [POS, D]
        nup_parts.append(r["out_nup"].reshape(POS))
        rem_parts.append(r["out_rem"].reshape(POS))
    new_state = np.concatenate(st_parts, 0).reshape(B, L, D).astype(np.float32)
    n_updates = np.concatenate(nup_parts, 0).reshape(B, L).astype(np.float32)
    remainders = np.concatenate(rem_parts, 0).reshape(B, L).astype(np.float32)
    return (new_state, (n_updates, remainders))


def kernel(**inputs):
    nc, _ = build_graph(float(np.float32(inputs["bp"][0])))
    res = run_bass_kernel_spmd(nc, prepare_in_maps(inputs), core_ids=list(range(NCORES)))
    return postprocess(res.results)


# revision 46
# speedup vs baseline: 1.0488x; 1.0488x over previous
"""ACT (Adaptive Computation Time) pondering network on 8 trn2 NeuronCores.

Data-parallel: 16384 positions sharded 2048/core; weights replicated.
All loop state SBUF-resident; fp32 matmuls (exact halting decisions vs the
f32 reference); halting chain replicates the reference's f32 op order.

Layout: activations transposed to [D, positions] so weights are matmul-
stationary and biases are per-partition scalars.

Toolchain constraint: each instruction may carry at most ONE semaphore wait
(+ its own update). All cross-engine deps funnel through DVE; startup
"observer" ops absorb DMA-queue semaphore ticks per engine.
"""
from contextlib import ExitStack
import numpy as np

import concourse.bass as bass
import concourse.tile as tile
from concourse import mybir
from concourse.bass_utils import run_bass_kernel_spmd

AF = mybir.ActivationFunctionType
OP = mybir.AluOpType
F32 = mybir.dt.float32


class SplitDrainTileContext(tile.TileContext):
    """Tile's kernel-tail drain collects one wait per proc (17 here) on a
    single instruction; this walrus build only encodes a couple of sync
    commands per instruction. Split the waits across standalone SP nops."""

    def _drain_and_barrier(self, tick_clock, wait_clock):
        from concourse.vector_clock import ScopedClock

        carrier = self.nc.sync.nop(nofuse=True)
        wait_clock.add_sem_waits(
            carrier.ins, ScopedClock({None: tick_clock.global_clock})
        )
        si = carrier.ins.sync_info
        waits = list(si.on_wait) if si and si.on_wait else []
        if len(waits) > 1:
            carrier.ins.sync_info = mybir.SyncInfo(
                on_wait=[waits[0]], on_update=list(si.on_update or [])
            )
            for w in waits[1:]:
                n = self.nc.sync.nop(nofuse=True)
                n.ins.sync_info = mybir.SyncInfo(on_wait=[w], on_update=[])
        # the nops above already hold SP until every proc reaches its final
        # tick, so the drain itself needs no waits
        self.nc.sync.drain()
        self.nc.all_engine_barrier()
        assert self.sems is not None
        popped = self.nc._tile_sem_poison_stack.pop()
        assert popped is self._sem_poison
        self.nc.clear_and_free_semaphores(list(self.sems.allocated().values()))
        self.nc.all_engine_barrier()

NCORES = 8
B, L, D, F = 4, 4096, 512, 2048
POS = B * L // NCORES          # 2048 positions per core
DC = D // 128                  # 4 d-chunks
FC = F // 128                  # 16 f-chunks
CH = 512                       # position chunk (one PSUM bank of f32)
PC = POS // CH                 # 4 position chunks
T = 11                         # MAX_ITERATIONS + 1
THR = float(np.float32(1.0 - 0.01))


def build_graph(bp0: float, n_iters: int = T):
    nc = bass.Bass()
    # const AP for the sigmoid bias (preamble, like Bass.__init__'s 0.0/1.0)
    _bp = nc.alloc_sbuf_tensor("const-bp0", [128, 1], F32)
    nc.gpsimd.memset(_bp.ap(), bp0)
    nc.const_aps.aps[(F32, bp0)] = _bp.ap()
    nc.all_engine_barrier()

    s0_ext = nc.declare_dram_parameter("s0", [DC, 128, POS], F32, isOutput=False)
    w1_ext = nc.declare_dram_parameter("w1", [DC, 128, F], F32, isOutput=False)
    w2_ext = nc.declare_dram_parameter("w2", [FC, 128, D], F32, isOutput=False)
    co_ext = nc.declare_dram_parameter("consts", [128, 688], F32, isOutput=False)
    mask_ext = nc.declare_dram_parameter("maskr", [PC, CH], F32, isOutput=False)
    ost_ext = nc.declare_dram_parameter("out_state", [DC, 128, POS], F32, isOutput=True)
    onu_ext = nc.declare_dram_parameter("out_nup", [PC, CH], F32, isOutput=True)
    ore_ext = nc.declare_dram_parameter("out_rem", [PC, CH], F32, isOutput=True)

    with SplitDrainTileContext(nc) as tc:
        with (
            tc.tile_pool(name="big", bufs=1) as big,
            tc.tile_pool(name="rows", bufs=1) as rows,
            tc.tile_pool(name="h1p", bufs=16) as h1p,
            tc.tile_pool(name="hwp", bufs=3) as hwp,
            tc.tile_pool(name="ps1", bufs=2, space="PSUM") as ps1,
            tc.tile_pool(name="ps2", bufs=2, space="PSUM") as ps2,
            tc.tile_pool(name="psp", bufs=2, space="PSUM") as psp,
            tc.tile_pool(name="psb", bufs=2, space="PSUM") as psb,
        ):
            # ---- persistent tiles + loads (one dma_start per tile) ----
            s = [big.tile([128, POS], F32, name=f"s{dc}", tag=f"s{dc}") for dc in range(DC)]
            w1 = [big.tile([128, F], F32, name=f"w1_{dc}", tag=f"w1_{dc}") for dc in range(DC)]
            w2 = [big.tile([128, D], F32, name=f"w2_{fc}", tag=f"w2_{fc}") for fc in range(FC)]
            prev = [big.tile([128, POS], F32, name=f"prev{dc}", tag=f"prev{dc}") for dc in range(DC)]
            uwb = big.tile([128, POS], F32, name="uwb", tag="uwb")
            # small constants share one host-packed tile: b1 @0:16, b2 @16:20,
            # emb @20:64, eye @64:576 (partitions 0..3), wpc @576:640,
            # scratch @640:688
            consts = rows.tile([128, 688], F32, name="consts", tag="consts")
            b1 = consts[:, 0:FC]
            b2 = consts[:, FC:FC + DC]
            _embo = FC + DC
            # halting-state rows live as [PC, CH] tiles (partition = position
            # chunk): every engine access starts at partition 0, and
            # two-input DVE ops see equal base partitions.
            def _r(name):
                return rows.tile([PC, CH], F32, name=name, tag=name)
            uw, hp, rem, nup = _r("uw"), _r("hp"), _r("rem"), _r("nup")
            pd, still, ps_ = _r("pd"), _r("still"), _r("ps_")
            q, nh, st2, t1 = _r("q"), _r("nh"), _r("st2"), _r("t1")
            prow_t = [_r("prow0"), _r("prow1"), _r("prow2")]  # rotates t%3
            maskr = _r("maskr")
            # aliases onto rows that are dead by the time these are written
            t3, u, u2 = still, q, ps_
            zrow = t1  # t1 fully consumed before the next iteration's z lands
            sacd = consts[0:1, 640:644]


            for dc in range(DC):
                nc.sync.dma_start(s[dc][:], s0_ext[dc])
                nc.sync.dma_start(w1[dc][:], w1_ext[dc])
            for fc in range(FC):
                nc.sync.dma_start(w2[fc][:], w2_ext[fc])
            nc.sync.dma_start(consts[:], co_ext[:])
            nc.sync.dma_start(maskr[:], mask_ext[:])


            nc.vector.memset(hp[:], 0.0)
            nc.vector.memset(rem[:], 0.0)
            nc.vector.memset(nup[:], 0.0)
            for dc in range(DC):
                nc.vector.memset(prev[dc][:], 0.0)

            # ---- startup observers: absorb DMA-queue sem ticks per engine ----
            # PE reads a 1x2 sliver of every matmul input so later matmuls
            # never need a DMA wait on top of their DVE wait.
            sac = psp.tile([1, 16], F32, name="sac", tag="pp")
            for tl in (*s, *w1, *w2, consts):
                nc.tensor.matmul(sac[:, 0:2], tl[0:1, 0:1], tl[0:1, 0:2],
                                 start=True, stop=True)
            # DVE touches the bias/mask tiles it will read mid-loop.
            nc.vector.tensor_copy(sacd[:, 0:1], b1[0:1, 0:1])
            nc.vector.tensor_copy(sacd[:, 1:2], maskr[0:1, 0:1])
            # ACT observes the consts DMA once so the per-iteration pre-reads
            # (which write into consts scratch) carry only the DVE wait.
            nc.scalar.copy(consts[0:1, 642:643], consts[0:1, 0:1])

            # guard registers: one set, reloaded each guarded iteration
            GUARD_FROM = 2
            # per-iteration alive scratch: unique addresses avoid Pool WAW waits
            # loop scratch lives in its own (never-DMA'd) tile so post-If
            # clock forks can't resurrect DMA-queue waits on its readers
            scr = rows.tile([4, 64], F32, name="scr", tag="scr")
            # reg-load targets live in their own tile: TensorLoad dependency
            # tracking is coarse, so writes to shared scratch would WAR them
            gscr = rows.tile([1, 32], F32, name="gscr", tag="gscr")
            def galive_w(tt):
                return gscr[0:1, tt:tt + 1]
            ones4 = scr[0:4, 50:51]
            cnt4 = scr[0:4, 48:49]

            nc.vector.memset(ones4, 1.0)
            # DVE observes the s-tile DMA queues (first DVE write to s is now
            # the epilogue s_next op, which must carry only the PE wait)
            for dc in range(DC):
                nc.vector.tensor_copy(scr[0:1, 44 + dc:45 + dc], s[dc][0:1, 0:1])
            regs = nc.alloc_registers(
                "alv", bass.OrderedSet([mybir.EngineType.PE, mybir.EngineType.DVE,
                                        mybir.EngineType.Activation]))

            # ---- the 11 ACT iterations ----
            TAIL_FROM = 5
            def emit_uw_bcast(t):
                # alive total + per-chunk counts for next iteration's guards
                pal = psb.tile([1, 1], F32, name=f"pal_{t}", tag="pb")
                nc.tensor.matmul(pal[:], ones4, cnt4, start=True, stop=True)
                nc.vector.tensor_copy(galive_w(t), pal[:])
                if t in (2, 3):
                    # per-chunk active counts feeding t+1's chunk guards;
                    # cnt4 (post-update) = exactly who participates at t+1.
                    # Unique gscr slot per t: TensorLoad dep tracking is coarse.
                    go = 16 + 4 * (t - 2)
                    pcr = psb.tile([1, PC], F32, name=f"pcr_{t}", tag="pb")
                    nc.tensor.matmul(pcr[:], cnt4, consts[0:PC, 680:680 + PC],
                                     start=True, stop=True)
                    nc.vector.tensor_copy(gscr[0:1, go:go + PC], pcr[:])
                # broadcast uw row j across partitions via eye-matmul
                for j in range(PC):
                    pb = psb.tile([128, CH], F32, name=f"pb_{t}_{j}", tag="pb")
                    nc.tensor.matmul(
                        pb[:], consts[0:PC, 64 + j * 128:64 + (j + 1) * 128],
                        uw[:], start=True, stop=True)
                    nc.vector.tensor_copy(uwb[:, j * CH:(j + 1) * CH], pb[:])

            def iter_body(t, absorb, chunk_guard=False):
                with nc.named_scope(f"iter{t}"):
                    if absorb:
                        # body-entry absorber: after an If, engine clocks fork
                        # conservatively; give DVE its ACT observation in one
                        # single-wait op before real work
                        nc.vector.tensor_copy(scr[0:1, 32 + t:33 + t],
                                              prow_t[(t - 1) % 3][0:1, 0:1])
                    # pondering: z = s . Wp  (fp32 exact): one [PC, CH] psum,
                    # row j from masked-Wp columns against position chunk j
                    pp = psp.tile([PC, CH], F32, name=f"pp_{t}", tag="pp")
                    for pc in range(PC):
                        for dc in range(DC):
                            wcol = 576 + (pc * DC + dc) * 4
                            nc.tensor.matmul(pp[:], consts[:, wcol:wcol + 4],
                                             s[dc][:, pc * CH:(pc + 1) * CH],
                                             start=(pc == 0 and dc == 0),
                                             stop=(pc == PC - 1 and dc == DC - 1))
                    nc.vector.tensor_copy(zrow[:], pp[:])
                    prow = prow_t[t % 3]
                    # ACT pre-read of one zrow element into a fresh scratch
                    # address: absorbs the DVE wait so the sigmoid carries only
                    # its own-engine (prow WAW) wait.
                    _sc = 644 + t
                    nc.scalar.copy(consts[0:1, _sc:_sc + 1], zrow[0:1, 0:1])
                    nc.scalar.activation(prow[:], zrow[:], AF.Sigmoid,
                                         bias=bp0, scale=1.0)
                    # halting chain; replicates reference f32 op order exactly
                    nc.vector.tensor_copy(pd[:], prow[:])          # import p to DVE
                    nc.vector.tensor_scalar(still[:], hp[:], 1.0, None, OP.is_lt)
                    nc.vector.tensor_mul(ps_[:], pd[:], still[:])
                    nc.vector.tensor_add(q[:], hp[:], ps_[:])
                    nc.vector.tensor_scalar(nh[:], q[:], THR, None, OP.is_gt)
                    nc.vector.tensor_mul(nh[:], nh[:], still[:])
                    nc.vector.tensor_sub(st2[:], still[:], nh[:])
                    # alive count for the next iteration's guard:
                    # DVE free-dim reduce, PE ones-matmul across the 4 chunks,
                    # DVE copy back -- each op carries one wait
                    nc.vector.tensor_reduce(cnt4, st2[:], axis=mybir.AxisListType.X,
                                            op=OP.add)
                    nc.vector.tensor_mul(t1[:], pd[:], st2[:])
                    nc.vector.tensor_add(hp[:], hp[:], t1[:])
                    nc.vector.tensor_scalar(u[:], hp[:], -1.0, 1.0, OP.mult, OP.add)
                    nc.vector.tensor_mul(u2[:], nh[:], u[:])
                    nc.vector.tensor_add(rem[:], rem[:], u2[:])
                    nc.vector.tensor_mul(t3[:], nh[:], rem[:])
                    nc.vector.tensor_add(hp[:], hp[:], t3[:])
                    nc.vector.tensor_add(nup[:], nup[:], st2[:])
                    nc.vector.tensor_add(nup[:], nup[:], nh[:])
                    nc.vector.tensor_add(uw[:], t1[:], t3[:])
                    nc.vector.tensor_mul(uw[:], uw[:], maskr[:])
                    if chunk_guard:
                        # per-chunk counts were computed at t-1; chunks with
                        # zero active positions contribute nothing (uw==0
                        # there), so skipping them is exact
                        emit_uw_bcast(t)
                    # FFN per position chunk
                    for pc in range(PC):
                        cstk = ExitStack()
                        if chunk_guard:
                            for reg in regs:
                                nc.reg_load(reg, gscr[0:1, 16 + 4 * (t - 3) + pc:17 + 4 * (t - 3) + pc]
                                            .bitcast(mybir.dt.int32))
                            cstk.enter_context(tc.If(nc.snap(regs) > 0, preferred_fallthrough_block=True))
                            # chunk-entry absorber: re-observe DVE inside the
                            # forked block with a single-wait op
                            nc.vector.tensor_copy(scr[0:1, 56 + pc:57 + pc],
                                                  uwb[0:1, pc * CH:pc * CH + 1])
                        sl = slice(pc * CH, (pc + 1) * CH)
                        h1 = []
                        for fc in range(FC):
                            pm = ps1.tile([128, CH], F32, name=f"pm_{t}_{pc}_{fc}", tag="pm")
                            for dc in range(DC):
                                nc.tensor.matmul(pm[:], w1[dc][:, fc * 128:(fc + 1) * 128],
                                                 s[dc][:, sl],
                                                 start=(dc == 0), stop=(dc == DC - 1))
                            h1t = h1p.tile([128, CH], F32, name=f"h1_{t}_{pc}_{fc}", tag="h1")
                            # h1 = max(pm + b1, 0)  (exact IEEE on DVE)
                            nc.vector.tensor_scalar(h1t[:], pm[:], b1[:, fc:fc + 1], 0.0,
                                                    OP.add, OP.max)
                            h1.append(h1t)
                        if pc == 0 and not chunk_guard:
                            # emitted here so these PE ops sit BEHIND the first
                            # mm1 block: the DVE halting chain they depend on
                            # finishes while mm1 streams
                            emit_uw_bcast(t)
                        for dt in range(DC):
                            pm2 = ps2.tile([128, CH], F32, name=f"pm2_{t}_{pc}_{dt}", tag="pm2")
                            for fc in range(FC):
                                nc.tensor.matmul(pm2[:], w2[fc][:, dt * 128:(dt + 1) * 128],
                                                 h1[fc][:], start=(fc == 0), stop=(fc == FC - 1))
                            if True:
                                # s_next first: the next iteration's pondering
                                # depends on it, so clear that edge early
                                ec = _embo + dt * T + min(t + 1, T - 1)
                                nc.vector.tensor_scalar(s[dt][:, sl], pm2[:],
                                                        b2[:, dt:dt + 1],
                                                        consts[:, ec:ec + 1],
                                                        OP.add, OP.add)
                            # prev += (pm2 + b2) * uw; h split from the product so
                            # each op carries a single semaphore wait; half-width
                            # temps to fit SBUF
                            for hf in range(2):
                                hsl = slice(pc * CH + hf * (CH // 2),
                                            pc * CH + (hf + 1) * (CH // 2))
                                psl = slice(hf * (CH // 2), (hf + 1) * (CH // 2))
                                ht = hwp.tile([128, CH // 2], F32,
                                              name=f"h_{t}_{pc}_{dt}_{hf}", tag="ht")
                                nc.vector.tensor_scalar(ht[:], pm2[:, psl],
                                                        b2[:, dt:dt + 1], None, OP.add)
                                hw = hwp.tile([128, CH // 2], F32,
                                              name=f"hw_{t}_{pc}_{dt}_{hf}", tag="hw")
                                nc.vector.tensor_mul(hw[:], ht[:], uwb[:, hsl])
                                nc.vector.tensor_add(prev[dt][:, hsl],
                                                     prev[dt][:, hsl], hw[:])
                        cstk.close()

            # unguarded warmup iterations, individually guarded middle, then
            # one guard over the whole tail (alive is monotone; a stale-true
            # tail just runs exact no-op iterations)
            for t in range(min(GUARD_FROM, n_iters)):
                iter_body(t, absorb=False)
            for t in range(GUARD_FROM, min(TAIL_FROM, n_iters)):
                with ExitStack() as stk:
                    for reg in regs:
                        # positive f32 bit patterns order like positive ints
                        nc.reg_load(reg, galive_w(t - 1).bitcast(mybir.dt.int32))
                    stk.enter_context(tc.If(nc.snap(regs) > 0, preferred_fallthrough_block=True))
                    iter_body(t, absorb=True, chunk_guard=(t >= 3))
            if n_iters > TAIL_FROM:
                with ExitStack() as stk:
                    for reg in regs:
                        nc.reg_load(reg, galive_w(TAIL_FROM - 1).bitcast(mybir.dt.int32))
                    stk.enter_context(tc.If(nc.snap(regs) > 0, preferred_fallthrough_block=True))
                    for t in range(TAIL_FROM, n_iters):
                        iter_body(t, absorb=(t == TAIL_FROM))

            # ---- outputs (gpsimd SWDGE: fresh queues, one wait each) ----
            for dc in range(DC):
                nc.gpsimd.dma_start(ost_ext[dc], prev[dc][:])
            nc.gpsimd.dma_start(onu_ext[:], nup[:])
            nc.gpsimd.dma_start(ore_ext[:], rem[:])

    return nc, tc


def check_waits(nc, verbose=True):
    """Static check: no instruction may carry more than one semaphore wait."""
    bad = 0
    for bb in nc.m.functions[0].blocks:
        for i in bb.instructions:
            si = i.sync_info
            nw = len(si.on_wait) if si and si.on_wait else 0
            if nw >= 2:
                bad += 1
                if verbose and bad <= 12:
                    print(f"MULTI-WAIT {type(i).__name__} {i.name}")
                    for w in si.on_wait:
                        print("   W:", str(w)[:100])
    return bad


def prepare_in_maps(inputs):
    state = np.asarray(inputs["state"], np.float32).reshape(-1, D)
    mask = np.asarray(inputs["mask"], np.float32).reshape(-1)
    emb = np.asarray(inputs["step_emb"], np.float32)
    Wp = np.asarray(inputs["Wp"], np.float32)
    W1 = np.asarray(inputs["W1"], np.float32)
    b1 = np.asarray(inputs["b1"], np.float32)
    W2 = np.asarray(inputs["W2"], np.float32)
    b2 = np.asarray(inputs["b2"], np.float32)

    w1t = np.ascontiguousarray(W1.reshape(DC, 128, F))
    w2t = np.ascontiguousarray(W2.reshape(FC, 128, D))

    # consts block: b1 @0:16, b2 @16:20, emb @20:64 (col = dt*T + t),
    # eye @64:576 (partition pc has ones in cols [pc*128,(pc+1)*128)),
    # wpc @576:640 (block (pc*DC+dc): col m==pc gets Wp[dc*128+k]),
    # scratch @640:688
    co = np.zeros((128, 688), np.float32)
    co[:, 0:FC] = b1.reshape(FC, 128).T
    co[:, FC:FC + DC] = b2.reshape(DC, 128).T
    co[:, FC + DC:FC + DC + DC * T] = emb.T.reshape(DC, 128, T).transpose(1, 0, 2).reshape(128, DC * T)
    for pc in range(PC):
        co[pc, 64 + pc * 128:64 + (pc + 1) * 128] = 1.0
    for pc in range(PC):
        for dc in range(DC):
            co[:, 576 + (pc * DC + dc) * 4 + pc] = Wp[dc * 128:(dc + 1) * 128, 0]
    co[0:4, 680:684] = np.eye(4, dtype=np.float32)

    # Sort each core's positions by p0 = sigmoid(s0 . Wp + bp) descending:
    # early halters cluster into chunks the per-chunk guards can skip.
    # Any permutation is exact (guards are value-driven); it only shifts rates.
    z0 = (state + emb[0]) @ Wp[:, 0] + float(inputs["bp"][0])
    p0 = 1.0 / (1.0 + np.exp(-z0))
    perms = []
    in_maps = []
    for c in range(NCORES):
        idx = np.argsort(-p0[c * POS:(c + 1) * POS], kind="stable")
        perms.append(idx)
        shard = state[c * POS:(c + 1) * POS][idx]                # [POS, D] permuted
        s0 = (shard.T + emb[0][:, None]).astype(np.float32)      # s_0 = state + emb[0]
        in_maps.append({
            "s0": np.ascontiguousarray(s0.reshape(DC, 128, POS)),
            "w1": w1t, "w2": w2t, "consts": co,
            "maskr": np.ascontiguousarray(
                mask[c * POS:(c + 1) * POS][idx].reshape(PC, CH)),
        })
    return in_maps, perms


def postprocess(results, perms):
    st_parts, nup_parts, rem_parts = [], [], []
    for r, idx in zip(results, perms):
        inv = np.empty_like(idx)
        inv[idx] = np.arange(idx.size)
        st_parts.append(r["out_state"].reshape(D, POS).T[inv])   # [POS, D]
        nup_parts.append(r["out_nup"].reshape(POS)[inv])
        rem_parts.append(r["out_rem"].reshape(POS)[inv])
    new_state = np.concatenate(st_parts, 0).reshape(B, L, D).astype(np.float32)
    n_updates = np.concatenate(nup_parts, 0).reshape(B, L).astype(np.float32)
    remainders = np.concatenate(rem_parts, 0).reshape(B, L).astype(np.float32)
    return (new_state, (n_updates, remainders))


def kernel(**inputs):
    nc, _ = build_graph(float(np.float32(inputs["bp"][0])))
    in_maps, perms = prepare_in_maps(inputs)
    res = run_bass_kernel_spmd(nc, in_maps, core_ids=list(range(NCORES)))
    return postprocess(res.results, perms)


# revision 47
# speedup vs baseline: 1.1618x; 1.1078x over previous
"""ACT (Adaptive Computation Time) pondering network on 8 trn2 NeuronCores.

Data-parallel: 16384 positions sharded 2048/core; weights replicated.
All loop state SBUF-resident; fp32 matmuls (exact halting decisions vs the
f32 reference); halting chain replicates the reference's f32 op order.

Layout: activations transposed to [D, positions] so weights are matmul-
stationary and biases are per-partition scalars.

Toolchain constraint: each instruction may carry at most ONE semaphore wait
(+ its own update). All cross-engine deps funnel through DVE; startup
"observer" ops absorb DMA-queue semaphore ticks per engine.
"""
from contextlib import ExitStack
import numpy as np

import concourse.bass as bass
import concourse.tile as tile
from concourse import mybir
from concourse.bass_utils import run_bass_kernel_spmd

AF = mybir.ActivationFunctionType
OP = mybir.AluOpType
F32 = mybir.dt.float32


class SplitDrainTileContext(tile.TileContext):
    """Tile's kernel-tail drain collects one wait per proc (17 here) on a
    single instruction; this walrus build only encodes a couple of sync
    commands per instruction. Split the waits across standalone SP nops."""

    def _drain_and_barrier(self, tick_clock, wait_clock):
        from concourse.vector_clock import ScopedClock

        carrier = self.nc.sync.nop(nofuse=True)
        wait_clock.add_sem_waits(
            carrier.ins, ScopedClock({None: tick_clock.global_clock})
        )
        si = carrier.ins.sync_info
        waits = list(si.on_wait) if si and si.on_wait else []
        if len(waits) > 1:
            carrier.ins.sync_info = mybir.SyncInfo(
                on_wait=[waits[0]], on_update=list(si.on_update or [])
            )
            for w in waits[1:]:
                n = self.nc.sync.nop(nofuse=True)
                n.ins.sync_info = mybir.SyncInfo(on_wait=[w], on_update=[])
        # the nops above already hold SP until every proc reaches its final
        # tick, so the drain itself needs no waits
        self.nc.sync.drain()
        self.nc.all_engine_barrier()
        assert self.sems is not None
        popped = self.nc._tile_sem_poison_stack.pop()
        assert popped is self._sem_poison
        self.nc.clear_and_free_semaphores(list(self.sems.allocated().values()))
        self.nc.all_engine_barrier()

NCORES = 8
B, L, D, F = 4, 4096, 512, 2048
POS = B * L // NCORES          # 2048 positions per core
DC = D // 128                  # 4 d-chunks
FC = F // 128                  # 16 f-chunks
CH = 512                       # position chunk (one PSUM bank of f32)
PC = POS // CH                 # 4 position chunks
T = 11                         # MAX_ITERATIONS + 1
THR = float(np.float32(1.0 - 0.01))


def build_graph(bp0: float, n_iters: int = T):
    nc = bass.Bass()
    # const AP for the sigmoid bias (preamble, like Bass.__init__'s 0.0/1.0)
    _bp = nc.alloc_sbuf_tensor("const-bp0", [128, 1], F32)
    nc.gpsimd.memset(_bp.ap(), bp0)
    nc.const_aps.aps[(F32, bp0)] = _bp.ap()
    nc.all_engine_barrier()

    s0_ext = nc.declare_dram_parameter("s0", [DC, 128, POS], F32, isOutput=False)
    w1_ext = nc.declare_dram_parameter("w1", [DC, 128, F], F32, isOutput=False)
    w2_ext = nc.declare_dram_parameter("w2", [FC, 128, D], F32, isOutput=False)
    co_ext = nc.declare_dram_parameter("consts", [128, 688], F32, isOutput=False)
    mask_ext = nc.declare_dram_parameter("maskr", [PC, CH], F32, isOutput=False)
    ost_ext = nc.declare_dram_parameter("out_state", [DC, 128, POS], F32, isOutput=True)
    onu_ext = nc.declare_dram_parameter("out_nup", [PC, CH], F32, isOutput=True)
    ore_ext = nc.declare_dram_parameter("out_rem", [PC, CH], F32, isOutput=True)

    with SplitDrainTileContext(nc) as tc:
        with (
            tc.tile_pool(name="big", bufs=1) as big,
            tc.tile_pool(name="rows", bufs=1) as rows,
            tc.tile_pool(name="h1p", bufs=16) as h1p,
            tc.tile_pool(name="hwp", bufs=3) as hwp,
            tc.tile_pool(name="ps1", bufs=2, space="PSUM") as ps1,
            tc.tile_pool(name="ps2", bufs=2, space="PSUM") as ps2,
            tc.tile_pool(name="psp", bufs=2, space="PSUM") as psp,
            tc.tile_pool(name="psb", bufs=2, space="PSUM") as psb,
        ):
            # ---- persistent tiles + loads (one dma_start per tile) ----
            s = [big.tile([128, POS], F32, name=f"s{dc}", tag=f"s{dc}") for dc in range(DC)]
            w1 = [big.tile([128, F], F32, name=f"w1_{dc}", tag=f"w1_{dc}") for dc in range(DC)]
            w2 = [big.tile([128, D], F32, name=f"w2_{fc}", tag=f"w2_{fc}") for fc in range(FC)]
            prev = [big.tile([128, POS], F32, name=f"prev{dc}", tag=f"prev{dc}") for dc in range(DC)]
            uwb = big.tile([128, POS], F32, name="uwb", tag="uwb")
            # small constants share one host-packed tile: b1 @0:16, b2 @16:20,
            # emb @20:64, eye @64:576 (partitions 0..3), wpc @576:640,
            # scratch @640:688
            consts = rows.tile([128, 688], F32, name="consts", tag="consts")
            b1 = consts[:, 0:FC]
            b2 = consts[:, FC:FC + DC]
            _embo = FC + DC
            # halting-state rows live as [PC, CH] tiles (partition = position
            # chunk): every engine access starts at partition 0, and
            # two-input DVE ops see equal base partitions.
            def _r(name):
                return rows.tile([PC, CH], F32, name=name, tag=name)
            uw, hp, rem, nup = _r("uw"), _r("hp"), _r("rem"), _r("nup")
            pd, still, ps_ = _r("pd"), _r("still"), _r("ps_")
            q, nh, st2, t1 = _r("q"), _r("nh"), _r("st2"), _r("t1")
            prow_t = [_r("prow0"), _r("prow1"), _r("prow2")]  # rotates t%3
            maskr = _r("maskr")
            # aliases onto rows that are dead by the time these are written
            t3, u, u2 = still, q, ps_
            zrow = t1  # t1 fully consumed before the next iteration's z lands
            sacd = consts[0:1, 640:644]


            for dc in range(DC):
                nc.sync.dma_start(s[dc][:], s0_ext[dc])
                nc.sync.dma_start(w1[dc][:], w1_ext[dc])
            for fc in range(FC):
                nc.sync.dma_start(w2[fc][:], w2_ext[fc])
            nc.sync.dma_start(consts[:], co_ext[:])
            nc.sync.dma_start(maskr[:], mask_ext[:])


            nc.vector.memset(hp[:], 0.0)
            nc.vector.memset(rem[:], 0.0)
            nc.vector.memset(nup[:], 0.0)
            for dc in range(DC):
                nc.vector.memset(prev[dc][:], 0.0)

            # ---- startup observers: absorb DMA-queue sem ticks per engine ----
            # PE reads a 1x2 sliver of every matmul input so later matmuls
            # never need a DMA wait on top of their DVE wait.
            sac = psp.tile([1, 16], F32, name="sac", tag="pp")
            for tl in (*s, *w1, *w2, consts):
                nc.tensor.matmul(sac[:, 0:2], tl[0:1, 0:1], tl[0:1, 0:2],
                                 start=True, stop=True)
            # DVE touches the bias/mask tiles it will read mid-loop.
            nc.vector.tensor_copy(sacd[:, 0:1], b1[0:1, 0:1])
            nc.vector.tensor_copy(sacd[:, 1:2], maskr[0:1, 0:1])
            # ACT observes the consts DMA once so the per-iteration pre-reads
            # (which write into consts scratch) carry only the DVE wait.
            nc.scalar.copy(consts[0:1, 642:643], consts[0:1, 0:1])

            # guard registers: one set, reloaded each guarded iteration
            GUARD_FROM = 2
            # per-iteration alive scratch: unique addresses avoid Pool WAW waits
            # loop scratch lives in its own (never-DMA'd) tile so post-If
            # clock forks can't resurrect DMA-queue waits on its readers
            scr = rows.tile([4, 64], F32, name="scr", tag="scr")
            # reg-load targets live in their own tile: TensorLoad dependency
            # tracking is coarse, so writes to shared scratch would WAR them
            gscr = rows.tile([1, 32], F32, name="gscr", tag="gscr")
            def galive_w(tt):
                return gscr[0:1, tt:tt + 1]
            ones4 = scr[0:4, 50:51]
            cnt4 = scr[0:4, 48:49]

            nc.vector.memset(ones4, 1.0)
            # DVE observes the s-tile DMA queues (first DVE write to s is now
            # the epilogue s_next op, which must carry only the PE wait)
            for dc in range(DC):
                nc.vector.tensor_copy(scr[0:1, 44 + dc:45 + dc], s[dc][0:1, 0:1])
            regs = nc.alloc_registers(
                "alv", bass.OrderedSet([mybir.EngineType.PE, mybir.EngineType.DVE,
                                        mybir.EngineType.Activation]))

            # ---- the 11 ACT iterations ----
            TAIL_FROM = 5
            def emit_uw_bcast(t):
                # alive total + per-chunk counts for next iteration's guards
                pal = psb.tile([1, 1], F32, name=f"pal_{t}", tag="pb")
                nc.tensor.matmul(pal[:], ones4, cnt4, start=True, stop=True)
                nc.vector.tensor_copy(galive_w(t), pal[:])
                if t in (1, 2, 3):
                    # per-chunk active counts feeding t+1's chunk guards;
                    # cnt4 (post-update) = exactly who participates at t+1.
                    # Unique gscr slot per t: TensorLoad dep tracking is coarse.
                    go = 16 + 4 * (t - 1)
                    pcr = psb.tile([1, PC], F32, name=f"pcr_{t}", tag="pb")
                    nc.tensor.matmul(pcr[:], cnt4, consts[0:PC, 680:680 + PC],
                                     start=True, stop=True)
                    nc.vector.tensor_copy(gscr[0:1, go:go + PC], pcr[:])
                # broadcast uw row j across partitions via eye-matmul
                for j in range(PC):
                    pb = psb.tile([128, CH], F32, name=f"pb_{t}_{j}", tag="pb")
                    nc.tensor.matmul(
                        pb[:], consts[0:PC, 64 + j * 128:64 + (j + 1) * 128],
                        uw[:], start=True, stop=True)
                    nc.vector.tensor_copy(uwb[:, j * CH:(j + 1) * CH], pb[:])

            def iter_body(t, absorb, chunk_guard=False):
                with nc.named_scope(f"iter{t}"):
                    if absorb:
                        # body-entry absorber: after an If, engine clocks fork
                        # conservatively; give DVE its ACT observation in one
                        # single-wait op before real work
                        nc.vector.tensor_copy(scr[0:1, 32 + t:33 + t],
                                              prow_t[(t - 1) % 3][0:1, 0:1])
                    # pondering: z = s . Wp  (fp32 exact): one [PC, CH] psum,
                    # row j from masked-Wp columns against position chunk j
                    pp = psp.tile([PC, CH], F32, name=f"pp_{t}", tag="pp")
                    for pc in range(PC):
                        for dc in range(DC):
                            wcol = 576 + (pc * DC + dc) * 4
                            nc.tensor.matmul(pp[:], consts[:, wcol:wcol + 4],
                                             s[dc][:, pc * CH:(pc + 1) * CH],
                                             start=(pc == 0 and dc == 0),
                                             stop=(pc == PC - 1 and dc == DC - 1))
                    nc.vector.tensor_copy(zrow[:], pp[:])
                    prow = prow_t[t % 3]
                    # ACT pre-read of one zrow element into a fresh scratch
                    # address: absorbs the DVE wait so the sigmoid carries only
                    # its own-engine (prow WAW) wait.
                    _sc = 644 + t
                    nc.scalar.copy(consts[0:1, _sc:_sc + 1], zrow[0:1, 0:1])
                    nc.scalar.activation(prow[:], zrow[:], AF.Sigmoid,
                                         bias=bp0, scale=1.0)
                    # halting chain; replicates reference f32 op order exactly
                    nc.vector.tensor_copy(pd[:], prow[:])          # import p to DVE
                    nc.vector.tensor_scalar(still[:], hp[:], 1.0, None, OP.is_lt)
                    nc.vector.tensor_mul(ps_[:], pd[:], still[:])
                    nc.vector.tensor_add(q[:], hp[:], ps_[:])
                    nc.vector.tensor_scalar(nh[:], q[:], THR, None, OP.is_gt)
                    nc.vector.tensor_mul(nh[:], nh[:], still[:])
                    nc.vector.tensor_sub(st2[:], still[:], nh[:])
                    # alive count for the next iteration's guard:
                    # DVE free-dim reduce, PE ones-matmul across the 4 chunks,
                    # DVE copy back -- each op carries one wait
                    nc.vector.tensor_reduce(cnt4, st2[:], axis=mybir.AxisListType.X,
                                            op=OP.add)
                    nc.vector.tensor_mul(t1[:], pd[:], st2[:])
                    nc.vector.tensor_add(hp[:], hp[:], t1[:])
                    nc.vector.tensor_scalar(u[:], hp[:], -1.0, 1.0, OP.mult, OP.add)
                    nc.vector.tensor_mul(u2[:], nh[:], u[:])
                    nc.vector.tensor_add(rem[:], rem[:], u2[:])
                    nc.vector.tensor_mul(t3[:], nh[:], rem[:])
                    nc.vector.tensor_add(hp[:], hp[:], t3[:])
                    nc.vector.tensor_add(nup[:], nup[:], st2[:])
                    nc.vector.tensor_add(nup[:], nup[:], nh[:])
                    nc.vector.tensor_add(uw[:], t1[:], t3[:])
                    nc.vector.tensor_mul(uw[:], uw[:], maskr[:])
                    if chunk_guard:
                        # per-chunk counts were computed at t-1; chunks with
                        # zero active positions contribute nothing (uw==0
                        # there), so skipping them is exact
                        emit_uw_bcast(t)
                    # FFN per position chunk
                    for pc in range(PC):
                        cstk = ExitStack()
                        if chunk_guard:
                            for reg in regs:
                                nc.reg_load(reg, gscr[0:1, 16 + 4 * (t - 2) + pc:17 + 4 * (t - 2) + pc]
                                            .bitcast(mybir.dt.int32))
                            cstk.enter_context(tc.If(nc.snap(regs) > 0, preferred_fallthrough_block=True))
                            # chunk-entry absorber: re-observe DVE inside the
                            # forked block with a single-wait op
                            nc.vector.tensor_copy(scr[0:1, 56 + pc:57 + pc],
                                                  uwb[0:1, pc * CH:pc * CH + 1])
                        sl = slice(pc * CH, (pc + 1) * CH)
                        h1 = []
                        for fc in range(FC):
                            pm = ps1.tile([128, CH], F32, name=f"pm_{t}_{pc}_{fc}", tag="pm")
                            for dc in range(DC):
                                nc.tensor.matmul(pm[:], w1[dc][:, fc * 128:(fc + 1) * 128],
                                                 s[dc][:, sl],
                                                 start=(dc == 0), stop=(dc == DC - 1))
                            h1t = h1p.tile([128, CH], F32, name=f"h1_{t}_{pc}_{fc}", tag="h1")
                            # h1 = max(pm + b1, 0)  (exact IEEE on DVE)
                            nc.vector.tensor_scalar(h1t[:], pm[:], b1[:, fc:fc + 1], 0.0,
                                                    OP.add, OP.max)
                            h1.append(h1t)
                        if pc == 0 and not chunk_guard:
                            # emitted here so these PE ops sit BEHIND the first
                            # mm1 block: the DVE halting chain they depend on
                            # finishes while mm1 streams
                            emit_uw_bcast(t)
                        for dt in range(DC):
                            pm2 = ps2.tile([128, CH], F32, name=f"pm2_{t}_{pc}_{dt}", tag="pm2")
                            for fc in range(FC):
                                nc.tensor.matmul(pm2[:], w2[fc][:, dt * 128:(dt + 1) * 128],
                                                 h1[fc][:], start=(fc == 0), stop=(fc == FC - 1))
                            if True:
                                # s_next first: the next iteration's pondering
                                # depends on it, so clear that edge early
                                ec = _embo + dt * T + min(t + 1, T - 1)
                                nc.vector.tensor_scalar(s[dt][:, sl], pm2[:],
                                                        b2[:, dt:dt + 1],
                                                        consts[:, ec:ec + 1],
                                                        OP.add, OP.add)
                            # prev += (pm2 + b2) * uw; h split from the product so
                            # each op carries a single semaphore wait; half-width
                            # temps to fit SBUF
                            for hf in range(2):
                                hsl = slice(pc * CH + hf * (CH // 2),
                                            pc * CH + (hf + 1) * (CH // 2))
                                psl = slice(hf * (CH // 2), (hf + 1) * (CH // 2))
                                ht = hwp.tile([128, CH // 2], F32,
                                              name=f"h_{t}_{pc}_{dt}_{hf}", tag="ht")
                                nc.vector.tensor_scalar(ht[:], pm2[:, psl],
                                                        b2[:, dt:dt + 1], None, OP.add)
                                hw = hwp.tile([128, CH // 2], F32,
                                              name=f"hw_{t}_{pc}_{dt}_{hf}", tag="hw")
                                nc.vector.tensor_mul(hw[:], ht[:], uwb[:, hsl])
                                nc.vector.tensor_add(prev[dt][:, hsl],
                                                     prev[dt][:, hsl], hw[:])
                        cstk.close()

            # unguarded warmup iterations, individually guarded middle, then
            # one guard over the whole tail (alive is monotone; a stale-true
            # tail just runs exact no-op iterations)
            for t in range(min(GUARD_FROM, n_iters)):
                iter_body(t, absorb=False)
            for t in range(GUARD_FROM, min(TAIL_FROM, n_iters)):
                with ExitStack() as stk:
                    for reg in regs:
                        # positive f32 bit patterns order like positive ints
                        nc.reg_load(reg, galive_w(t - 1).bitcast(mybir.dt.int32))
                    stk.enter_context(tc.If(nc.snap(regs) > 0, preferred_fallthrough_block=True))
                    iter_body(t, absorb=True, chunk_guard=True)
            if n_iters > TAIL_FROM:
                with ExitStack() as stk:
                    for reg in regs:
                        nc.reg_load(reg, galive_w(TAIL_FROM - 1).bitcast(mybir.dt.int32))
                    stk.enter_context(tc.If(nc.snap(regs) > 0, preferred_fallthrough_block=True))
                    for t in range(TAIL_FROM, n_iters):
                        iter_body(t, absorb=(t == TAIL_FROM))

            # ---- outputs (gpsimd SWDGE: fresh queues, one wait each) ----
            for dc in range(DC):
                nc.gpsimd.dma_start(ost_ext[dc], prev[dc][:])
            nc.gpsimd.dma_start(onu_ext[:], nup[:])
            nc.gpsimd.dma_start(ore_ext[:], rem[:])

    return nc, tc


def check_waits(nc, verbose=True):
    """Static check: no instruction may carry more than one semaphore wait."""
    bad = 0
    for bb in nc.m.functions[0].blocks:
        for i in bb.instructions:
            si = i.sync_info
            nw = len(si.on_wait) if si and si.on_wait else 0
            if nw >= 2:
                bad += 1
                if verbose and bad <= 12:
                    print(f"MULTI-WAIT {type(i).__name__} {i.name}")
                    for w in si.on_wait:
                        print("   W:", str(w)[:100])
    return bad


def prepare_in_maps(inputs):
    state = np.asarray(inputs["state"], np.float32).reshape(-1, D)
    mask = np.asarray(inputs["mask"], np.float32).reshape(-1)
    emb = np.asarray(inputs["step_emb"], np.float32)
    Wp = np.asarray(inputs["Wp"], np.float32)
    W1 = np.asarray(inputs["W1"], np.float32)
    b1 = np.asarray(inputs["b1"], np.float32)
    W2 = np.asarray(inputs["W2"], np.float32)
    b2 = np.asarray(inputs["b2"], np.float32)

    w1t = np.ascontiguousarray(W1.reshape(DC, 128, F))
    w2t = np.ascontiguousarray(W2.reshape(FC, 128, D))

    # consts block: b1 @0:16, b2 @16:20, emb @20:64 (col = dt*T + t),
    # eye @64:576 (partition pc has ones in cols [pc*128,(pc+1)*128)),
    # wpc @576:640 (block (pc*DC+dc): col m==pc gets Wp[dc*128+k]),
    # scratch @640:688
    co = np.zeros((128, 688), np.float32)
    co[:, 0:FC] = b1.reshape(FC, 128).T
    co[:, FC:FC + DC] = b2.reshape(DC, 128).T
    co[:, FC + DC:FC + DC + DC * T] = emb.T.reshape(DC, 128, T).transpose(1, 0, 2).reshape(128, DC * T)
    for pc in range(PC):
        co[pc, 64 + pc * 128:64 + (pc + 1) * 128] = 1.0
    for pc in range(PC):
        for dc in range(DC):
            co[:, 576 + (pc * DC + dc) * 4 + pc] = Wp[dc * 128:(dc + 1) * 128, 0]
    co[0:4, 680:684] = np.eye(4, dtype=np.float32)

    # Sort each core's positions by a host-computed halt predictor: positions
    # with q1 = p0 + p1 > threshold halt by t=1 (exactly); the rest sort by q1
    # descending so late halters cluster into chunks the per-chunk guards can
    # skip. Any permutation is exact (guards are value-driven); it only
    # shifts skip rates. Costs ~0.5s of numpy (one FFN pass contracted
    # through W2.Wp).
    bp0f = float(inputs["bp"][0])
    s0f = state + emb[0]
    p0 = 1.0 / (1.0 + np.exp(-((s0f @ Wp[:, 0]) + bp0f)))
    w2wp = W2.astype(np.float32) @ Wp[:, 0]
    z1 = (np.maximum(s0f @ W1.astype(np.float32) + b1, 0) @ w2wp
          + (b2 + emb[1]) @ Wp[:, 0] + bp0f)
    q1 = p0 + 1.0 / (1.0 + np.exp(-z1))
    key = np.where(q1 > 0.99, 2.0 + q1, q1)
    perms = []
    in_maps = []
    for c in range(NCORES):
        idx = np.argsort(-key[c * POS:(c + 1) * POS], kind="stable")
        perms.append(idx)
        shard = state[c * POS:(c + 1) * POS][idx]                # [POS, D] permuted
        s0 = (shard.T + emb[0][:, None]).astype(np.float32)      # s_0 = state + emb[0]
        in_maps.append({
            "s0": np.ascontiguousarray(s0.reshape(DC, 128, POS)),
            "w1": w1t, "w2": w2t, "consts": co,
            "maskr": np.ascontiguousarray(
                mask[c * POS:(c + 1) * POS][idx].reshape(PC, CH)),
        })
    return in_maps, perms


def postprocess(results, perms):
    st_parts, nup_parts, rem_parts = [], [], []
    for r, idx in zip(results, perms):
        inv = np.empty_like(idx)
        inv[idx] = np.arange(idx.size)
        st_parts.append(r["out_state"].reshape(D, POS).T[inv])   # [POS, D]
        nup_parts.append(r["out_nup"].reshape(POS)[inv])
        rem_parts.append(r["out_rem"].reshape(POS)[inv])
    new_state = np.concatenate(st_parts, 0).reshape(B, L, D).astype(np.float32)
    n_updates = np.concatenate(nup_parts, 0).reshape(B, L).astype(np.float32)
    remainders = np.concatenate(rem_parts, 0).reshape(B, L).astype(np.float32)
    return (new_state, (n_updates, remainders))


def kernel(**inputs):
    nc, _ = build_graph(float(np.float32(inputs["bp"][0])))
    in_maps, perms = prepare_in_maps(inputs)
    res = run_bass_kernel_spmd(nc, in_maps, core_ids=list(range(NCORES)))
    return postprocess(res.results, perms)
